# revision 7
# baseline (speedup 1.0000x reference)
"""Bass/Trainium2 kernel for a single LSTM-cell step + tiny MLP head.

Reference computation (all fp32):
    gates = W_ih @ x + b_ih + W_hh @ h0 + b_hh        # [4H], gate order i,f,g,o
    i, f, g, o = sigmoid/sigmoid/tanh/sigmoid splits
    c = f * c0 + i * g ; h = o * tanh(c)              # [H]
    z = relu(W1 @ h + b1)                             # [32]
    out = sigmoid(W2 @ z + b2)                        # [130]

Fast path (used when h0 == 0 and c0 == 0, which holds for this model's
inputs): W_hh @ h0 == 0 so the W_hh stream is skipped entirely, and
f * c0 == 0 so the f-gate rows of W_ih are never loaded either. Each of
the 8 cores owns hidden slice s_k = [k*512, (k+1)*512) and streams only
the [i | g | o] rows of W_ih for its slice -- a [1536, 8197] matrix with
the bias folded in via a constant-1 element appended to x.

Weights are stored *128 in fp8e4m3 (the scale keeps values out of the
subnormal range; the gate activations descale by 1/128 for free via the
activation unit's scale operand). Matmuls run in DoubleRow perf mode
(two K-tiles per instruction) so the fp8 stream is DMA-bound, not
PE-bound. The LSTM epilogue runs locally; h is re-tiled to partition-
major via 4 PE transposes (no DRAM round-trip); the partial MLP dot
z_part = W1[:, s_k] @ h_k -> [32] is AllReduce'd (tiny) and every core
finishes the replicated MLP head with b1/b2 folded into the activation
bias / an extra constant-1 row.

Dummy AllReduce(s) issued at kernel start pay the one-time collective
bootstrap (entry barrier + cold-op cost) underneath the weight stream.
Dummy matmuls on resident SBUF data pad each DMA group's PE work so the
PE never idles (idle gaps drop it to half clock).

Inputs with nonzero h0/c0 take a numpy fallback that evaluates the
exact reference math on the host, so kernel() stays correct for
arbitrary inputs.
"""

import os

import numpy as np
import ml_dtypes

D = 8196
H = 4096
HS = 512            # hidden slice per core
R = 3 * HS          # gate rows per core: [i | g | o] (f dropped: c0 == 0)
HID = 32
OUT = 130
NCORES = 8
MMN = 512           # matmul free dim = one PSUM bank
NB = R // MMN       # 3
S = 128.0           # fp8 weight pre-scale; descaled in the gate activations

K1D = D + 1         # x ++ 1.0 (bias column)
K1T = 65            # ceil(8197/128) K-tiles
K1P = K1T * 128

MREP = int(os.environ.get("KERNEL_MREP", "16"))   # stationary col replication
G = int(os.environ.get("KERNEL_G", "2"))          # DoubleRow pairs per group
WBUFS = int(os.environ.get("KERNEL_BUFS", "6"))
NDUMCC = int(os.environ.get("KERNEL_NDUMCC", "1"))
DUMMY = os.environ.get("KERNEL_DUMMY", "auto")    # HAM-warm pad per group
BWGBS = float(os.environ.get("KERNEL_BW", "345"))  # assumed DMA GB/s for pad
STAGE = os.environ.get("KERNEL_STAGE", "full")    # debug: "h" | "z" | "full"

FP8 = ml_dtypes.float8_e4m3fn
_cached = {}


def _groups():
    """Group sizes in K-tiles (even = all DoubleRow pairs; a small ramp
    first so the PE starts early; odd remainder rides in the last group)."""
    gk = 2 * G
    sizes = [2, 2]
    rem = K1T - sum(sizes)
    sizes += [gk] * (rem // gk)
    if rem % gk:
        sizes.append(rem % gk)
    return sizes


def build_nc():
    """Build + compile the per-core Bass program (same program on all cores)."""
    import concourse.tile as tile
    from concourse import bacc, mybir

    fp32 = mybir.dt.float32
    bf16 = mybir.dt.bfloat16
    fp8 = mybir.dt.float8e4
    AF = mybir.ActivationFunctionType
    DR = mybir.MatmulPerfMode.DoubleRow

    nc = bacc.Bacc("TRN2", target_bir_lowering=False, debug=False,
                   num_devices=NCORES)

    wt_d = nc.dram_tensor("wt", [128, K1T, R], fp8, kind="ExternalInput")
    xt_d = nc.dram_tensor("xt", [128, K1T, MREP], fp8, kind="ExternalInput")
    w1_d = nc.dram_tensor("w1t", [128, HS // 128, HID], bf16,
                          kind="ExternalInput")
    b1_d = nc.dram_tensor("b1", [HID], fp32, kind="ExternalInput")
    w2_d = nc.dram_tensor("w2e", [HID + 1, OUT], fp32, kind="ExternalInput")
    out_d = nc.dram_tensor("out", [OUT], fp32, kind="ExternalOutput")

    zp_d = nc.dram_tensor("zpart", [HID], fp32)
    zr_d = nc.dram_tensor("zred", [HID], fp32, addr_space="Shared")
    dum_d = nc.dram_tensor("ccdummy", [HID], fp32)
    dumr_d = nc.dram_tensor("ccdummyr", [HID], fp32, addr_space="Shared")

    GK = 2 * G
    group_sizes = _groups()

    with tile.TileContext(nc) as tc:
        with (
            tc.tile_pool(name="weights", bufs=WBUFS) as wpool,
            tc.tile_pool(name="small", bufs=1) as small,
            tc.tile_pool(name="psg", bufs=1, space="PSUM") as psg,
            tc.tile_pool(name="psd", bufs=1, space="PSUM") as psd,
            tc.tile_pool(name="pst", bufs=1, space="PSUM") as pst,
            tc.tile_pool(name="psz", bufs=1, space="PSUM") as psz,
            tc.tile_pool(name="pso", bufs=1, space="PSUM") as pso,
        ):
            # dummy collective(s) first: pay the CC entry barrier + cold-op
            # latency underneath the weight stream so the real AllReduce
            # later runs warm. The dummy's (all-zero) result is added into
            # z below, which pins the dummy ahead of the real AllReduce in
            # the compile-time CC stream order -- without the data dep the
            # scheduler can slot the real one first, making it run cold and
            # leaving the dummy dangling past the end of the program.
            dumr_sb = None
            if STAGE == "full":
                zt = small.tile([1, HID], fp32)
                nc.gpsimd.memset(zt[:], 0.0)
                nc.gpsimd.dma_start(dum_d[None, :], zt[:])
                for _ in range(NDUMCC):
                    nc.gpsimd.collective_compute(
                        "AllReduce",
                        mybir.AluOpType.add,
                        replica_groups=[list(range(NCORES))],
                        ins=[dum_d[:]],
                        outs=[dumr_d[:]],
                    )
                if NDUMCC:
                    dumr_sb = small.tile([1, HID], fp32)
                    nc.scalar.dma_start(dumr_sb[:], dumr_d[None, :])

            # small persistent operands on the scalar HWDGE ring (the sync
            # ring is reserved for the weight stream)
            xt_sb = small.tile([128, K1T, MREP], fp8)
            nc.scalar.dma_start(xt_sb[:], xt_d[:])
            w1_sb = small.tile([128, HS // 128, HID], bf16)
            nc.scalar.dma_start(w1_sb[:], w1_d[:])
            b1_sb = small.tile([HID, 1], fp32)
            nc.scalar.dma_start(b1_sb[:], b1_d[:, None])
            w2_sb = small.tile([HID + 1, OUT], fp32)
            nc.scalar.dma_start(w2_sb[:], w2_d[:])

            # resident garbage operands + scratch PSUM bank for PE-warming
            # dummy matmuls; identity scalar for the PE transposes
            dmy_x = small.tile([128, 2, MREP], fp8)
            nc.gpsimd.memset(dmy_x[:], 0.0)
            dmy_w = small.tile([128, 2, MMN], fp8)
            nc.gpsimd.memset(dmy_w[:], 0.0)
            dmy_ps = psd.tile([MREP, MMN], fp32)
            ident = small.tile([1, 1], fp32)
            nc.gpsimd.memset(ident[:], 1.0)

            # one PSUM tile per gate bank so each activation's dependency
            # resolves as soon as its own bank's accumulation closes
            gates_ps = [psg.tile([MREP, MMN], fp32, tag=f"g{nb}",
                                 name=f"gates{nb}") for nb in range(NB)]

            kk = 0          # global K-tile index for start/stop flags
            g0 = 0
            for gs in group_sizes:
                wtile = wpool.tile([128, GK, R], fp8, tag="w")
                nc.sync.dma_start(wtile[:, :gs, :], wt_d[:, g0:g0 + gs, :])
                for j in range(gs // 2):
                    t = g0 + 2 * j
                    for nb in range(NB):
                        nc.tensor.matmul(
                            gates_ps[nb][:],
                            lhsT=xt_sb[:, t:t + 2, :],
                            rhs=wtile[:, 2 * j:2 * j + 2,
                                      nb * MMN:(nb + 1) * MMN],
                            start=(kk == 0),
                            stop=(kk + 2 == K1T),
                            perf_mode=DR,
                        )
                    kk += 2
                if gs % 2:
                    t = g0 + gs - 1
                    for nb in range(NB):
                        nc.tensor.matmul(
                            gates_ps[nb][:],
                            lhsT=xt_sb[:, t, :],
                            rhs=wtile[:, gs - 1, nb * MMN:(nb + 1) * MMN],
                            start=(kk == 0),
                            stop=(kk + 1 == K1T),
                        )
                    kk += 1
                # pad PE work up to the group's DMA time so the PE never
                # idles (idle gaps drop it to half clock)
                if DUMMY == "auto":
                    dma_ns = gs * R * 128 / BWGBS * 1e0
                    pe_ns = (gs // 2) * NB * 250 + (gs % 2) * NB * 230
                    ndum = max(0, int((dma_ns - pe_ns) / 260))
                else:
                    ndum = int(DUMMY)
                for _ in range(ndum):
                    nc.tensor.matmul(dmy_ps[:], lhsT=dmy_x[:], rhs=dmy_w[:],
                                     start=True, stop=True, perf_mode=DR)
                g0 += gs

            # LSTM epilogue on partition 0: gates_ps row 0 = [i | g | o],
            # all values scaled by S -- the activation descales via `scale`
            sc = 1.0 / S
            i_sb = small.tile([1, HS], fp32)
            nc.scalar.activation(i_sb[:], gates_ps[0][0:1, :],
                                 AF.Sigmoid, scale=sc)
            g_sb = small.tile([1, HS], fp32)
            nc.scalar.activation(g_sb[:], gates_ps[1][0:1, :],
                                 AF.Tanh, scale=sc)
            o_sb = small.tile([1, HS], fp32)
            nc.scalar.activation(o_sb[:], gates_ps[2][0:1, :],
                                 AF.Sigmoid, scale=sc)
            c_sb = small.tile([1, HS], fp32)
            nc.vector.tensor_mul(c_sb[:], i_sb[:], g_sb[:])
            tch = small.tile([1, HS], fp32)
            nc.scalar.activation(tch[:], c_sb[:], AF.Tanh)
            h_sb = small.tile([1, HS], fp32)
            nc.vector.tensor_mul(h_sb[:], o_sb[:], tch[:])

            if STAGE == "h":
                nc.scalar.dma_start(out_d[None, :], h_sb[0:1, :OUT])
            else:
                # re-tile h [1,512] -> [128,4] partition-major via 4 PE
                # transposes (matmul writes only clear the accumulate bits
                # of the shared bank, never prior columns' data)
                hT_ps = pst.tile([128, HS // 128], fp32)
                for t in range(HS // 128):
                    nc.tensor.transpose(hT_ps[:, t:t + 1],
                                        h_sb[0:1, t * 128:(t + 1) * 128],
                                        ident[:])
                hT_sb = small.tile([128, HS // 128], bf16)
                nc.vector.tensor_copy(hT_sb[:], hT_ps[:])

                # partial MLP layer 1: z_part = W1[:, s_k] @ h_k -> [32]
                z_ps = psz.tile([1, HID], fp32)
                for t in range(HS // 128):
                    nc.tensor.matmul(z_ps[:], lhsT=hT_sb[:, t:t + 1],
                                     rhs=w1_sb[:, t, :],
                                     start=(t == 0), stop=(t == HS // 128 - 1))
                z_sb = small.tile([1, HID], fp32)
                if dumr_sb is not None and STAGE == "full":
                    # + 0: the dummy AllReduce result is all zeros; the add
                    # only exists to order the CC stream (see above)
                    nc.vector.tensor_add(z_sb[:], z_ps[0:1, :], dumr_sb[:])
                else:
                    nc.vector.tensor_copy(z_sb[:], z_ps[0:1, :])

                if STAGE == "z":
                    nc.scalar.dma_start(out_d[None, :HID], z_sb[:])
                else:
                    nc.scalar.dma_start(zp_d[None, :], z_sb[:])
                    nc.gpsimd.collective_compute(
                        "AllReduce",
                        mybir.AluOpType.add,
                        replica_groups=[list(range(NCORES))],
                        ins=[zp_d[:]],
                        outs=[zr_d[:]],
                    )
                    # reload reduced z as [32,1] (partition-per-element);
                    # relu folds the +b1 via the activation bias operand
                    zr_sb = small.tile([HID, 1], fp32)
                    nc.scalar.dma_start(zr_sb[:], zr_d[:, None])
                    zrelu = small.tile([HID + 1, 1], fp32)
                    nc.gpsimd.memset(zrelu[:], 1.0)   # row 32 stays 1.0
                    nc.scalar.activation(zrelu[0:HID, :], zr_sb[:],
                                         AF.Relu, bias=b1_sb[:])
                    # w2e row 32 = b2, so sigmoid applies directly on PSUM
                    out_ps = pso.tile([1, OUT], fp32)
                    nc.tensor.matmul(out_ps[:], lhsT=zrelu[:], rhs=w2_sb[:],
                                     start=True, stop=True)
                    res = small.tile([1, OUT], fp32)
                    nc.scalar.activation(res[:], out_ps[0:1, :], AF.Sigmoid)
                    nc.scalar.dma_start(out_d[None, :], res[:])

    nc.compile()
    return nc


def get_nc():
    if "nc" not in _cached:
        _cached["nc"] = build_nc()
    return _cached["nc"]


def shard_inputs(inputs):
    """Slice/transpose/scale/cast the full inputs into per-core input maps."""
    x = np.asarray(inputs["x"], np.float32)
    W_ih = np.asarray(inputs["W_ih"], np.float32)
    b = (np.asarray(inputs["b_ih"], np.float32)
         + np.asarray(inputs["b_hh"], np.float32))
    W1 = np.asarray(inputs["W1"], np.float32)
    b1 = np.asarray(inputs["b1"], np.float32)
    W2 = np.asarray(inputs["W2"], np.float32)
    b2 = np.asarray(inputs["b2"], np.float32)

    xc = np.zeros(K1P, np.float32)
    xc[:D] = x
    xc[D] = 1.0
    # xt[p, t, m] = xc[t*128 + p], replicated over m
    xt = np.ascontiguousarray(
        np.repeat(xc.reshape(K1T, 128).T[:, :, None], MREP, axis=2)
    ).astype(FP8)

    w2e = np.ascontiguousarray(
        np.concatenate([W2.T, b2[None, :]], axis=0))          # [33, 130]

    in_maps = []
    for k in range(NCORES):
        rows = np.concatenate([np.arange(g * H + k * HS, g * H + (k + 1) * HS)
                               for g in (0, 2, 3)])           # i, g, o rows
        Wf = np.zeros((R, K1P), np.float32)
        Wf[:, :D] = W_ih[rows] * S
        Wf[:, D] = b[rows] * S
        # wt[p, t, j] = Wf[j, t*128 + p]
        wt = np.ascontiguousarray(
            Wf.T.reshape(K1T, 128, R).transpose(1, 0, 2)).astype(FP8)
        # w1t[p, t, j] = W1[j, k*HS + t*128 + p]
        w1t = np.ascontiguousarray(
            W1[:, k * HS:(k + 1) * HS].T
            .reshape(HS // 128, 128, HID).transpose(1, 0, 2)
        ).astype(ml_dtypes.bfloat16)
        in_maps.append({
            "wt": wt,
            "xt": xt,
            "w1t": w1t,
            "b1": b1,
            "w2e": w2e,
        })
    return in_maps


def _reference_numpy(inputs):
    """Exact reference math on the host (general-inputs fallback)."""
    f32 = {k: np.asarray(v, np.float32) for k, v in inputs.items()}
    gates = (f32["W_ih"] @ f32["x"] + f32["b_ih"]
             + f32["W_hh"] @ f32["h0"] + f32["b_hh"])
    i, f, g, o = np.split(gates, 4)
    i = 1.0 / (1.0 + np.exp(-i))
    f = 1.0 / (1.0 + np.exp(-f))
    g = np.tanh(g)
    o = 1.0 / (1.0 + np.exp(-o))
    c = f * f32["c0"] + i * g
    h = o * np.tanh(c)
    z = np.maximum(f32["W1"] @ h + f32["b1"], 0.0)
    u = f32["W2"] @ z + f32["b2"]
    return (1.0 / (1.0 + np.exp(-u))).astype(np.float32)


def run(inputs, trace=False):
    from concourse.bass_utils import run_bass_kernel_spmd
    nc = get_nc()
    in_maps = shard_inputs(inputs)
    return run_bass_kernel_spmd(nc, in_maps, list(range(NCORES)), trace=trace)


def kernel(**inputs) -> np.ndarray:
    h0 = np.asarray(inputs["h0"])
    c0 = np.asarray(inputs["c0"])
    if np.any(h0 != 0) or np.any(c0 != 0):
        # fast path assumes h0 == c0 == 0 (drops W_hh and the f gate);
        # general inputs fall back to the exact host computation
        return _reference_numpy(inputs)
    res = run(inputs, trace=False)
    return np.asarray(res.results[0]["out"], np.float32)


# revision 10
# speedup vs baseline: 1.0581x; 1.0581x over previous
"""Bass/Trainium2 kernel for a single LSTM-cell step + tiny MLP head.

Reference computation (all fp32):
    gates = W_ih @ x + b_ih + W_hh @ h0 + b_hh        # [4H], gate order i,f,g,o
    i, f, g, o = sigmoid/sigmoid/tanh/sigmoid splits
    c = f * c0 + i * g ; h = o * tanh(c)              # [H]
    z = relu(W1 @ h + b1)                             # [32]
    out = sigmoid(W2 @ z + b2)                        # [130]

Fast path (used when h0 == 0 and c0 == 0, which holds for this model's
inputs): W_hh @ h0 == 0 so the W_hh stream is skipped entirely, and
f * c0 == 0 so the f-gate rows of W_ih are never loaded either. Each of
the 8 cores owns hidden slice s_k = [k*512, (k+1)*512) and streams only
the [i | g | o] rows of W_ih for its slice -- a [1536, 8197] matrix with
the bias folded in via a constant-1 element appended to x.

Weights are stored *128 in fp8e4m3 (the scale keeps values out of the
subnormal range; the gate activations descale by 1/128 for free via the
activation unit's scale operand). Matmuls run in DoubleRow perf mode
(two K-tiles per instruction) so the fp8 stream is DMA-bound, not
PE-bound. The LSTM epilogue runs locally; h is re-tiled to partition-
major via 4 PE transposes (no DRAM round-trip); the partial MLP dot
z_part = W1[:, s_k] @ h_k -> [32] is AllReduce'd (tiny) and every core
finishes the replicated MLP head with b1/b2 folded into the activation
bias / an extra constant-1 row.

Dummy AllReduce(s) issued at kernel start pay the one-time collective
bootstrap (entry barrier + cold-op cost) underneath the weight stream.
Dummy matmuls on resident SBUF data pad each DMA group's PE work so the
PE never idles (idle gaps drop it to half clock).

Inputs with nonzero h0/c0 take a numpy fallback that evaluates the
exact reference math on the host, so kernel() stays correct for
arbitrary inputs.
"""

import os

import numpy as np
import ml_dtypes

D = 8196
H = 4096
HS = 512            # hidden slice per core
R = 3 * HS          # gate rows per core: [i | g | o] (f dropped: c0 == 0)
HID = 32
OUT = 130
NCORES = 8
MMN = 512           # matmul free dim = one PSUM bank
NB = R // MMN       # 3
S = 128.0           # fp8 weight pre-scale; descaled in the gate activations

K1D = D + 1         # x ++ 1.0 (bias column)
K1T = 65            # ceil(8197/128) K-tiles
K1P = K1T * 128

MREP = int(os.environ.get("KERNEL_MREP", "16"))   # stationary col replication
G = int(os.environ.get("KERNEL_G", "2"))          # DoubleRow pairs per group
WBUFS = int(os.environ.get("KERNEL_BUFS", "6"))
NDUMCC = int(os.environ.get("KERNEL_NDUMCC", "1"))
DUMMY = os.environ.get("KERNEL_DUMMY", "auto")    # HAM-warm pad per group
BWGBS = float(os.environ.get("KERNEL_BW", "345"))  # assumed DMA GB/s for pad
STAGE = os.environ.get("KERNEL_STAGE", "full")    # debug: "h" | "z" | "full"

FP8 = ml_dtypes.float8_e4m3fn
_cached = {}


def _groups():
    """Group sizes in K-tiles (even = all DoubleRow pairs; a small ramp
    first so the PE starts early; odd remainder rides in the last group)."""
    gk = 2 * G
    sizes = [2, 2]
    rem = K1T - sum(sizes)
    sizes += [gk] * (rem // gk)
    if rem % gk:
        sizes.append(rem % gk)
    return sizes


def build_nc():
    """Build + compile the per-core Bass program (same program on all cores)."""
    import concourse.tile as tile
    from concourse import bacc, mybir

    fp32 = mybir.dt.float32
    bf16 = mybir.dt.bfloat16
    fp8 = mybir.dt.float8e4
    AF = mybir.ActivationFunctionType
    DR = mybir.MatmulPerfMode.DoubleRow

    nc = bacc.Bacc("TRN2", target_bir_lowering=False, debug=False,
                   num_devices=NCORES)

    wt_d = nc.dram_tensor("wt", [128, K1T, R], fp8, kind="ExternalInput")
    xt_d = nc.dram_tensor("xt", [128, K1T, MREP], fp8, kind="ExternalInput")
    w1_d = nc.dram_tensor("w1t", [128, HS // 128, HID], bf16,
                          kind="ExternalInput")
    b1_d = nc.dram_tensor("b1", [HID], fp32, kind="ExternalInput")
    w2_d = nc.dram_tensor("w2e", [HID + 1, OUT], fp32, kind="ExternalInput")
    out_d = nc.dram_tensor("out", [OUT], fp32, kind="ExternalOutput")

    zp_d = nc.dram_tensor("zpart", [HID], fp32)
    zr_d = nc.dram_tensor("zred", [HID], fp32, addr_space="Shared")
    dum_d = nc.dram_tensor("ccdummy", [HID], fp32)
    dumr_d = nc.dram_tensor("ccdummyr", [HID], fp32)

    GK = 2 * G
    group_sizes = _groups()

    with tile.TileContext(nc) as tc:
        with (
            tc.tile_pool(name="weights", bufs=WBUFS) as wpool,
            tc.tile_pool(name="small", bufs=1) as small,
            tc.tile_pool(name="psg", bufs=1, space="PSUM") as psg,
            tc.tile_pool(name="psd", bufs=1, space="PSUM") as psd,
            tc.tile_pool(name="pst", bufs=1, space="PSUM") as pst,
            tc.tile_pool(name="psz", bufs=1, space="PSUM") as psz,
            tc.tile_pool(name="pso", bufs=1, space="PSUM") as pso,
        ):
            # Dummy collective first. Its ONLY purpose is to fire a CC
            # doorbell as early as possible: the one-time collective entry
            # barrier (~33-41us) starts at the FIRST doorbell arrival,
            # regardless of stream slot order. The scheduler sinks the
            # consumer-less dummy to the LAST stream slot, so the real
            # AllReduce runs first (right after the barrier) and the dummy
            # trails past the output path -- hence 2-rank groups to keep
            # that trailing op as short as possible.
            if STAGE == "full":
                zt = small.tile([1, HID], fp32)
                nc.gpsimd.memset(zt[:], 0.0)
                nc.gpsimd.dma_start(dum_d[None, :], zt[:])
                dum_groups = [[2 * i, 2 * i + 1] for i in range(NCORES // 2)]
                for _ in range(NDUMCC):
                    nc.gpsimd.collective_compute(
                        "AllReduce",
                        mybir.AluOpType.add,
                        replica_groups=dum_groups,
                        ins=[dum_d[:]],
                        outs=[dumr_d[:]],
                    )

            # small persistent operands on the scalar HWDGE ring (the sync
            # ring is reserved for the weight stream)
            xt_sb = small.tile([128, K1T, MREP], fp8)
            nc.scalar.dma_start(xt_sb[:], xt_d[:])
            w1_sb = small.tile([128, HS // 128, HID], bf16)
            nc.scalar.dma_start(w1_sb[:], w1_d[:])
            b1_sb = small.tile([HID, 1], fp32)
            nc.scalar.dma_start(b1_sb[:], b1_d[:, None])
            w2_sb = small.tile([HID + 1, OUT], fp32)
            nc.scalar.dma_start(w2_sb[:], w2_d[:])

            # resident garbage operands + scratch PSUM bank for PE-warming
            # dummy matmuls; identity scalar for the PE transposes
            dmy_x = small.tile([128, 2, MREP], fp8)
            nc.gpsimd.memset(dmy_x[:], 0.0)
            dmy_w = small.tile([128, 2, MMN], fp8)
            nc.gpsimd.memset(dmy_w[:], 0.0)
            dmy_ps = psd.tile([MREP, MMN], fp32)
            ident = small.tile([1, 1], fp32)
            nc.gpsimd.memset(ident[:], 1.0)

            # one PSUM tile per gate bank so each activation's dependency
            # resolves as soon as its own bank's accumulation closes
            gates_ps = [psg.tile([MREP, MMN], fp32, tag=f"g{nb}",
                                 name=f"gates{nb}") for nb in range(NB)]

            kk = 0          # global K-tile index for start/stop flags
            g0 = 0
            for gs in group_sizes:
                wtile = wpool.tile([128, GK, R], fp8, tag="w")
                nc.sync.dma_start(wtile[:, :gs, :], wt_d[:, g0:g0 + gs, :])
                for j in range(gs // 2):
                    t = g0 + 2 * j
                    for nb in range(NB):
                        nc.tensor.matmul(
                            gates_ps[nb][:],
                            lhsT=xt_sb[:, t:t + 2, :],
                            rhs=wtile[:, 2 * j:2 * j + 2,
                                      nb * MMN:(nb + 1) * MMN],
                            start=(kk == 0),
                            stop=(kk + 2 == K1T),
                            perf_mode=DR,
                        )
                    kk += 2
                if gs % 2:
                    t = g0 + gs - 1
                    for nb in range(NB):
                        nc.tensor.matmul(
                            gates_ps[nb][:],
                            lhsT=xt_sb[:, t, :],
                            rhs=wtile[:, gs - 1, nb * MMN:(nb + 1) * MMN],
                            start=(kk == 0),
                            stop=(kk + 1 == K1T),
                        )
                    kk += 1
                # pad PE work up to the group's DMA time so the PE never
                # idles (idle gaps drop it to half clock)
                if DUMMY == "auto":
                    dma_ns = gs * R * 128 / BWGBS * 1e0
                    pe_ns = (gs // 2) * NB * 250 + (gs % 2) * NB * 230
                    ndum = max(0, int((dma_ns - pe_ns) / 260))
                else:
                    ndum = int(DUMMY)
                for _ in range(ndum):
                    nc.tensor.matmul(dmy_ps[:], lhsT=dmy_x[:], rhs=dmy_w[:],
                                     start=True, stop=True, perf_mode=DR)
                g0 += gs

            # LSTM epilogue on partition 0: gates_ps row 0 = [i | g | o],
            # all values scaled by S -- the activation descales via `scale`
            sc = 1.0 / S
            i_sb = small.tile([1, HS], fp32)
            nc.scalar.activation(i_sb[:], gates_ps[0][0:1, :],
                                 AF.Sigmoid, scale=sc)
            g_sb = small.tile([1, HS], fp32)
            nc.scalar.activation(g_sb[:], gates_ps[1][0:1, :],
                                 AF.Tanh, scale=sc)
            o_sb = small.tile([1, HS], fp32)
            nc.scalar.activation(o_sb[:], gates_ps[2][0:1, :],
                                 AF.Sigmoid, scale=sc)
            c_sb = small.tile([1, HS], fp32)
            nc.vector.tensor_mul(c_sb[:], i_sb[:], g_sb[:])
            tch = small.tile([1, HS], fp32)
            nc.scalar.activation(tch[:], c_sb[:], AF.Tanh)
            h_sb = small.tile([1, HS], fp32)
            nc.vector.tensor_mul(h_sb[:], o_sb[:], tch[:])

            if STAGE == "h":
                nc.scalar.dma_start(out_d[None, :], h_sb[0:1, :OUT])
            else:
                # re-tile h [1,512] -> [128,4] partition-major via 4 PE
                # transposes (matmul writes only clear the accumulate bits
                # of the shared bank, never prior columns' data)
                hT_ps = pst.tile([128, HS // 128], fp32)
                for t in range(HS // 128):
                    nc.tensor.transpose(hT_ps[:, t:t + 1],
                                        h_sb[0:1, t * 128:(t + 1) * 128],
                                        ident[:])
                hT_sb = small.tile([128, HS // 128], bf16)
                nc.vector.tensor_copy(hT_sb[:], hT_ps[:])

                # partial MLP layer 1: z_part = W1[:, s_k] @ h_k -> [32]
                z_ps = psz.tile([1, HID], fp32)
                for t in range(HS // 128):
                    nc.tensor.matmul(z_ps[:], lhsT=hT_sb[:, t:t + 1],
                                     rhs=w1_sb[:, t, :],
                                     start=(t == 0), stop=(t == HS // 128 - 1))
                z_sb = small.tile([1, HID], fp32)
                nc.vector.tensor_copy(z_sb[:], z_ps[0:1, :])

                if STAGE == "z":
                    nc.scalar.dma_start(out_d[None, :HID], z_sb[:])
                else:
                    nc.scalar.dma_start(zp_d[None, :], z_sb[:])
                    nc.gpsimd.collective_compute(
                        "AllReduce",
                        mybir.AluOpType.add,
                        replica_groups=[list(range(NCORES))],
                        ins=[zp_d[:]],
                        outs=[zr_d[:]],
                    )
                    # reload reduced z as [32,1] (partition-per-element);
                    # relu folds the +b1 via the activation bias operand
                    zr_sb = small.tile([HID, 1], fp32)
                    nc.scalar.dma_start(zr_sb[:], zr_d[:, None])
                    zrelu = small.tile([HID + 1, 1], fp32)
                    nc.gpsimd.memset(zrelu[:], 1.0)   # row 32 stays 1.0
                    nc.scalar.activation(zrelu[0:HID, :], zr_sb[:],
                                         AF.Relu, bias=b1_sb[:])
                    # w2e row 32 = b2, so sigmoid applies directly on PSUM
                    out_ps = pso.tile([1, OUT], fp32)
                    nc.tensor.matmul(out_ps[:], lhsT=zrelu[:], rhs=w2_sb[:],
                                     start=True, stop=True)
                    res = small.tile([1, OUT], fp32)
                    nc.scalar.activation(res[:], out_ps[0:1, :], AF.Sigmoid)
                    nc.scalar.dma_start(out_d[None, :], res[:])

    nc.compile()
    return nc


def get_nc():
    if "nc" not in _cached:
        _cached["nc"] = build_nc()
    return _cached["nc"]


def shard_inputs(inputs):
    """Slice/transpose/scale/cast the full inputs into per-core input maps."""
    x = np.asarray(inputs["x"], np.float32)
    W_ih = np.asarray(inputs["W_ih"], np.float32)
    b = (np.asarray(inputs["b_ih"], np.float32)
         + np.asarray(inputs["b_hh"], np.float32))
    W1 = np.asarray(inputs["W1"], np.float32)
    b1 = np.asarray(inputs["b1"], np.float32)
    W2 = np.asarray(inputs["W2"], np.float32)
    b2 = np.asarray(inputs["b2"], np.float32)

    xc = np.zeros(K1P, np.float32)
    xc[:D] = x
    xc[D] = 1.0
    # xt[p, t, m] = xc[t*128 + p], replicated over m
    xt = np.ascontiguousarray(
        np.repeat(xc.reshape(K1T, 128).T[:, :, None], MREP, axis=2)
    ).astype(FP8)

    w2e = np.ascontiguousarray(
        np.concatenate([W2.T, b2[None, :]], axis=0))          # [33, 130]

    in_maps = []
    for k in range(NCORES):
        rows = np.concatenate([np.arange(g * H + k * HS, g * H + (k + 1) * HS)
                               for g in (0, 2, 3)])           # i, g, o rows
        Wf = np.zeros((R, K1P), np.float32)
        Wf[:, :D] = W_ih[rows] * S
        Wf[:, D] = b[rows] * S
        # wt[p, t, j] = Wf[j, t*128 + p]
        wt = np.ascontiguousarray(
            Wf.T.reshape(K1T, 128, R).transpose(1, 0, 2)).astype(FP8)
        # w1t[p, t, j] = W1[j, k*HS + t*128 + p]
        w1t = np.ascontiguousarray(
            W1[:, k * HS:(k + 1) * HS].T
            .reshape(HS // 128, 128, HID).transpose(1, 0, 2)
        ).astype(ml_dtypes.bfloat16)
        in_maps.append({
            "wt": wt,
            "xt": xt,
            "w1t": w1t,
            "b1": b1,
            "w2e": w2e,
        })
    return in_maps


def _reference_numpy(inputs):
    """Exact reference math on the host (general-inputs fallback)."""
    f32 = {k: np.asarray(v, np.float32) for k, v in inputs.items()}
    gates = (f32["W_ih"] @ f32["x"] + f32["b_ih"]
             + f32["W_hh"] @ f32["h0"] + f32["b_hh"])
    i, f, g, o = np.split(gates, 4)
    i = 1.0 / (1.0 + np.exp(-i))
    f = 1.0 / (1.0 + np.exp(-f))
    g = np.tanh(g)
    o = 1.0 / (1.0 + np.exp(-o))
    c = f * f32["c0"] + i * g
    h = o * np.tanh(c)
    z = np.maximum(f32["W1"] @ h + f32["b1"], 0.0)
    u = f32["W2"] @ z + f32["b2"]
    return (1.0 / (1.0 + np.exp(-u))).astype(np.float32)


def run(inputs, trace=False):
    from concourse.bass_utils import run_bass_kernel_spmd
    nc = get_nc()
    in_maps = shard_inputs(inputs)
    return run_bass_kernel_spmd(nc, in_maps, list(range(NCORES)), trace=trace)


def kernel(**inputs) -> np.ndarray:
    h0 = np.asarray(inputs["h0"])
    c0 = np.asarray(inputs["c0"])
    if np.any(h0 != 0) or np.any(c0 != 0):
        # fast path assumes h0 == c0 == 0 (drops W_hh and the f gate);
        # general inputs fall back to the exact host computation
        return _reference_numpy(inputs)
    res = run(inputs, trace=False)
    return np.asarray(res.results[0]["out"], np.float32)
